# revision 21
# baseline (speedup 1.0000x reference)
"""Trainium2 Bass kernel for nn_Block (sliding-window MHA + top-2 MoE).

Sharding: data-parallel over batch across 8 NeuronCores (4 batches/core),
weights replicated, no collectives. Each core runs the full block on its
2048-token shard.

Numerics: the top-2 expert routing is a discrete selection — any upstream
error beyond ~1e-5 flips choices vs the reference and produces large
per-token errors. So everything that feeds the router (LN1, attention,
proj, LN2, gate) runs in full fp32; only the post-gate expert FFNs (smooth
in their inputs) run in bf16 for 4x PE throughput.

Layout: activations transposed to feature-major [C, tok] via PE transposes
so matmuls contract over the partition dim. Attention is banded (WIN=128):
per 128-query tile only the diagonal + left 128-key tiles are computed.
"""

import numpy as np

B, T, C = 32, 512, 512
H, D = 8, 64
E, F = 4, 2048
WIN = 128
EPS = 1e-5
NCORES = 8
BPC = B // NCORES          # batches per core
NTOK = BPC * T             # tokens per core
P = 128
CO = C // P                # contraction chunks over C
FO = F // P                # F tiles
QT = T // P                # query tiles per batch
SCALE = float(C) ** -0.5

_CACHE = {}


def _build(n_batches):
    import contextlib
    import os

    # bisect knobs: PHASE 1=LN1/QKV only, 2=+attention, 3=+proj/LN2/gate,
    # 4=full; ACCUM=0 replaces the DRAM accumulate DMA with a plain write
    PHASE = int(os.environ.get("K_PHASE", "4"))
    ACCUM = os.environ.get("K_ACCUM", "1") == "1"
    GATE = os.environ.get("K_GATE", "1") == "1"
    TTR = os.environ.get("K_TTR", "0") == "1"  # ttr hangs on HW

    import concourse.bacc as bacc
    import concourse.mybir as mybir
    import concourse.tile as tile
    from concourse.masks import make_identity

    f32 = mybir.dt.float32
    bf16 = mybir.dt.bfloat16
    AF = mybir.ActivationFunctionType
    AX = mybir.AxisListType
    OP = mybir.AluOpType

    ntok = n_batches * T
    tt_n = ntok // P
    tch_n = ntok // 512

    nc = bacc.Bacc("TRN2", target_bir_lowering=False, debug=False)

    def din(name, shape, dt=f32):
        return nc.dram_tensor(name, list(shape), dt, kind="ExternalInput").ap()

    x_d = din("x", (ntok, C))
    ln1_g = din("ln1_g", (C,))
    ln1_b = din("ln1_b", (C,))
    wq_d = din("Wq", (H, C, D))
    wk_d = din("Wk", (H, C, D))
    wv_d = din("Wv", (H, C, D))
    wp_d = din("Wp", (C, C))
    bp_d = din("bp", (C,))
    ln2_g = din("ln2_g", (C,))
    ln2_b = din("ln2_b", (C,))
    wgT_d = din("WgT", (E, C))
    bg_d = din("bg", (E,))
    w1_d = din("W1", (E, C, F), bf16)
    b1_d = din("b1", (E, F))
    w2_d = din("W2", (E, F, C), bf16)
    b2_d = din("b2", (E, C))
    mask_d = din("mask", (P, 2 * P))
    out_d = nc.dram_tensor("out", [ntok, C], f32, kind="ExternalOutput").ap()

    with tile.TileContext(nc) as tc, contextlib.ExitStack() as top:
        singles = top.enter_context(tc.tile_pool(name="singles", bufs=1))
        small = top.enter_context(tc.tile_pool(name="small", bufs=4))
        hpool = top.enter_context(tc.tile_pool(name="hpool", bufs=3))
        ps_mm = top.enter_context(tc.tile_pool(name="ps_mm", bufs=2, space="PSUM"))

        # ---- constants ----
        ident32 = singles.tile([P, P], f32, tag="id32")
        make_identity(nc, ident32)
        eps_t = singles.tile([P, 1], f32, tag="eps")
        nc.vector.memset(eps_t, EPS)
        mask_t = singles.tile([P, 2 * P], f32, tag="mask")
        nc.sync.dma_start(mask_t, mask_d)

        def bcast(name, ap, n):
            t = singles.tile([P, n], f32, tag=name)
            nc.sync.dma_start(t, ap[None, :].to_broadcast((P, n)))
            return t

        ln1g_t = bcast("ln1g", ln1_g, C)
        ln1b_t = bcast("ln1b", ln1_b, C)
        ln2g_t = bcast("ln2g", ln2_g, C)
        ln2b_t = bcast("ln2b", ln2_b, C)
        bp_t = bcast("bp", bp_d, C)
        bg_t = bcast("bg", bg_d, E)
        # Wg columns broadcast across partitions for the DVE gate reduce
        wgc_t = singles.tile([P, E, C], f32, tag="wgc")
        nc.sync.dma_start(wgc_t, wgT_d[None].to_broadcast((P, E, C)))

        def emit_ln(x_ap, g_t, b_t):
            """LayerNorm of [128, C] slice -> new [128, C] f32 tile."""
            st = small.tile([P, 6], f32, tag="bnst")
            nc.vector.bn_stats(out=st, in_=x_ap)
            mv = small.tile([P, 2], f32, tag="bnmv")
            nc.vector.bn_aggr(out=mv, in_=st)
            sq = small.tile([P, 1], f32, tag="sq")
            nc.scalar.activation(sq, mv[:, 1:2], AF.Sqrt, bias=eps_t)
            rstd = small.tile([P, 1], f32, tag="rstd")
            nc.vector.reciprocal(rstd, sq)
            nmr = small.tile([P, 1], f32, tag="nmr")
            nc.vector.tensor_mul(nmr, mv[:, 0:1], rstd)
            nc.vector.tensor_scalar_mul(nmr, nmr, -1.0)
            hh = hpool.tile([P, C], f32, tag="h")
            nc.scalar.activation(hh, x_ap, AF.Identity, bias=nmr, scale=rstd)
            nc.vector.tensor_mul(hh, hh, g_t)
            nc.vector.tensor_add(hh, hh, b_t)
            return hh

        # =============== attention ===============
        with contextlib.ExitStack() as attn:
            apool = attn.enter_context(tc.tile_pool(name="attn", bufs=1))
            wstream = attn.enter_context(tc.tile_pool(name="wstream", bufs=1))
            expp = attn.enter_context(tc.tile_pool(name="expp", bufs=3))
            trp = attn.enter_context(tc.tile_pool(name="trp", bufs=3))
            ps_sc = attn.enter_context(
                tc.tile_pool(name="ps_sc", bufs=2, space="PSUM")
            )
            ps_tr = attn.enter_context(
                tc.tile_pool(name="ps_tr", bufs=2, space="PSUM")
            )
            ps_ov = attn.enter_context(
                tc.tile_pool(name="ps_ov", bufs=2, space="PSUM")
            )

            # hT (f32 feature-major); oT later reuses the same slot
            hT = apool.tile([P, CO, ntok], f32, tag="hT_oT")

            # ---- LN1 + transpose ----
            for tt in range(tt_n):
                xs = hpool.tile([P, C], f32, tag="xs")
                nc.sync.dma_start(xs, x_d[tt * P : (tt + 1) * P, :])
                hh = emit_ln(xs, ln1g_t, ln1b_t)
                for co in range(CO):
                    tp = ps_mm.tile([P, 512], f32, tag="mm512")
                    nc.tensor.transpose(
                        tp[:, :P], hh[:, co * P : (co + 1) * P], ident32
                    )
                    nc.vector.tensor_copy(hT[:, co, tt * P : (tt + 1) * P], tp[:, :P])

            # ---- QKV (fp32) ----
            qT = apool.tile([P, CO, ntok], f32, tag="qT")
            kT = apool.tile([P, CO, ntok], f32, tag="kT")
            for wd, dstT in ((wq_d, qT), (wk_d, kT)):
                wsb = wstream.tile([P, CO, H, D], f32, tag="wqkv")
                for co in range(CO):
                    nc.sync.dma_start(
                        wsb[:, co],
                        wd.rearrange("h (co ci) d -> ci co h d", ci=P)[:, co],
                    )
                for fo in range(CO):
                    for ch in range(tch_n):
                        mm = ps_mm.tile([P, 512], f32, tag="mm512")
                        for co in range(CO):
                            nc.tensor.matmul(
                                mm,
                                wsb[:, co, 2 * fo : 2 * fo + 2, :],
                                hT[:, co, ch * 512 : (ch + 1) * 512],
                                start=(co == 0),
                                stop=(co == CO - 1),
                            )
                        nc.vector.tensor_copy(
                            dstT[:, fo, ch * 512 : (ch + 1) * 512], mm
                        )
            v_sb = apool.tile([P, tt_n, C], f32, tag="v")
            wvsb = wstream.tile([P, CO, H, D], f32, tag="wqkv")
            for co in range(CO):
                nc.sync.dma_start(
                    wvsb[:, co],
                    wv_d.rearrange("h (co ci) d -> ci co h d", ci=P)[:, co],
                )
            for tt in range(tt_n):
                mm = ps_mm.tile([P, 512], f32, tag="mm512")
                for co in range(CO):
                    nc.tensor.matmul(
                        mm,
                        hT[:, co, tt * P : (tt + 1) * P],
                        wvsb[:, co],
                        start=(co == 0),
                        stop=(co == CO - 1),
                    )
                nc.vector.tensor_copy(v_sb[:, tt, :], mm)

            if PHASE < 2:
                nc.sync.dma_start(out_d[0:P, :], v_sb[:, 0, :])
                nc.sync.dma_start(out_d[P : 2 * P, :], qT[:, 0, 0:C])
                nc.sync.dma_start(out_d[2 * P : 3 * P, :], kT[:, 0, 0:C])

            # ---- banded attention ----
            oT = apool.tile([P, CO, ntok], f32, tag="hT_oT")
            for b in range(n_batches if PHASE >= 2 else 0):
                base = b * T
                for h in range(H):
                    hp0 = 64 * (h % 2)
                    fo_h = h // 2
                    for qt in range(QT):
                        tw = 2 * P if qt > 0 else P
                        ts0 = base + (qt - 1) * P if qt > 0 else base
                        sc = ps_sc.tile([P, 2 * P], f32, tag="sc")
                        nc.tensor.matmul(
                            sc[:, :tw],
                            qT[hp0 : hp0 + 64, fo_h,
                               base + qt * P : base + (qt + 1) * P],
                            kT[hp0 : hp0 + 64, fo_h, ts0 : ts0 + tw],
                            start=True,
                            stop=True,
                        )
                        exps = expp.tile([P, 2 * P], f32, tag="exps")
                        nc.scalar.activation(
                            exps[:, :tw], sc[:, :tw], AF.Exp, scale=SCALE
                        )
                        nc.vector.tensor_mul(
                            exps[:, :tw], exps[:, :tw], mask_t[:, 2 * P - tw :]
                        )
                        red = small.tile([P, 1], f32, tag="sred")
                        nc.vector.reduce_sum(out=red, in_=exps[:, :tw], axis=AX.X)
                        rec = small.tile([P, 1], f32, tag="srec")
                        nc.vector.reciprocal(rec, red)
                        probs = expp.tile([P, 2 * P], f32, tag="probs")
                        nc.scalar.activation(
                            probs[:, :tw], exps[:, :tw], AF.Copy, scale=rec
                        )
                        ov = ps_ov.tile([P, P], f32, tag="ov", name="ov")[:64]
                        nh = tw // P
                        for half in range(nh):
                            kt = (qt - 1 + half) if qt > 0 else 0
                            tp = ps_tr.tile([P, P], f32, tag="ptr")
                            nc.tensor.transpose(
                                tp, probs[:, half * P : (half + 1) * P], ident32
                            )
                            pT = trp.tile([P, P], f32, tag="pT")
                            nc.vector.tensor_copy(pT, tp)
                            nc.tensor.matmul(
                                ov,
                                v_sb[:, b * QT + kt, h * D : (h + 1) * D],
                                pT,
                                start=(half == 0),
                                stop=(half == nh - 1),
                            )
                        nc.vector.tensor_copy(
                            oT[hp0 : hp0 + 64, fo_h,
                               base + qt * P : base + (qt + 1) * P],
                            ov,
                        )

            if PHASE == 2:
                nc.sync.dma_start(out_d[0:P, :], oT[:, 0, 0:C])

            # ---- proj + residual -> x2, staged in out_d (DRAM) ----
            for tt in range(tt_n if PHASE >= 3 else 0):
                if tt == 0:
                    wpsb = wstream.tile([P, CO, C], f32, tag="wp")
                    nc.sync.dma_start(
                        wpsb, wp_d.rearrange("(fo fi) c -> fi fo c", fi=P)
                    )
                mm = ps_mm.tile([P, 512], f32, tag="mm512")
                for fo in range(CO):
                    nc.tensor.matmul(
                        mm,
                        oT[:, fo, tt * P : (tt + 1) * P],
                        wpsb[:, fo],
                        start=(fo == 0),
                        stop=(fo == CO - 1),
                    )
                xs = hpool.tile([P, C], f32, tag="xs")
                nc.sync.dma_start(xs, x_d[tt * P : (tt + 1) * P, :])
                x2t = hpool.tile([P, C], f32, tag="x2t")
                nc.vector.tensor_add(x2t, xs, mm)
                nc.vector.tensor_add(x2t, x2t, bp_t)
                nc.sync.dma_start(out_d[tt * P : (tt + 1) * P, :], x2t)

        # =============== MoE ===============
        with contextlib.ExitStack() as moe:
            mpool = moe.enter_context(tc.tile_pool(name="moe", bufs=1))
            wpool = moe.enter_context(tc.tile_pool(name="moew", bufs=2))
            hidp = moe.enter_context(tc.tile_pool(name="hidp", bufs=2))
            tmpp = moe.enter_context(tc.tile_pool(name="tmpp", bufs=3))
            ps_eo = moe.enter_context(
                tc.tile_pool(name="ps_eo", bufs=2, space="PSUM")
            )

            # ---- LN2 + transpose (bf16) + gate/top-2 ----
            h2T = mpool.tile([P, CO, ntok], bf16, tag="h2T")
            w_sb = mpool.tile([P, tt_n, E], f32, tag="w")
            for tt in range(tt_n if (PHASE >= 3 and GATE) else 0):
                xs2 = hpool.tile([P, C], f32, tag="xs2")
                nc.sync.dma_start(xs2, out_d[tt * P : (tt + 1) * P, :])
                hh = emit_ln(xs2, ln2g_t, ln2b_t)
                for co in range(CO):
                    tp = ps_mm.tile([P, 512], f32, tag="mm512")
                    nc.tensor.transpose(
                        tp[:, :P], hh[:, co * P : (co + 1) * P], ident32
                    )
                    nc.vector.tensor_copy(
                        h2T[:, co, tt * P : (tt + 1) * P], tp[:, :P]
                    )
                # gate logits in f32 on DVE: gate[:, e] = h2 . Wg[:, e] + bg[e]
                gate = small.tile([P, E], f32, tag="gate")
                for e in range(E):
                    scr = tmpp.tile([P, C], f32, tag="ttr")
                    if TTR:
                        nc.vector.tensor_tensor_reduce(
                            out=scr,
                            in0=hh,
                            in1=wgc_t[:, e],
                            scale=1.0,
                            scalar=bg_t[:, e : e + 1],
                            op0=OP.mult,
                            op1=OP.add,
                            accum_out=gate[:, e : e + 1],
                        )
                    else:
                        nc.vector.tensor_mul(scr, hh, wgc_t[:, e])
                        nc.vector.reduce_sum(
                            out=gate[:, e : e + 1], in_=scr, axis=AX.X
                        )
                        nc.vector.tensor_add(
                            gate[:, e : e + 1],
                            gate[:, e : e + 1],
                            bg_t[:, e : e + 1],
                        )
                # top-2 of 4 + softmax weights via max / is_ge / sigmoid
                m1 = small.tile([P, 1], f32, tag="m1")
                nc.vector.reduce_max(out=m1, in_=gate, axis=AX.X)
                is1 = small.tile([P, E], f32, tag="is1")
                nc.vector.tensor_tensor(
                    is1, gate, m1.to_broadcast((P, E)), OP.is_ge
                )
                big = small.tile([P, E], f32, tag="big")
                nc.vector.tensor_scalar_mul(big, is1, 1e30)
                gate2 = small.tile([P, E], f32, tag="gate2")
                nc.vector.tensor_sub(gate2, gate, big)
                m2 = small.tile([P, 1], f32, tag="m2")
                nc.vector.reduce_max(out=m2, in_=gate2, axis=AX.X)
                is2 = small.tile([P, E], f32, tag="is2")
                nc.vector.tensor_tensor(
                    is2, gate2, m2.to_broadcast((P, E)), OP.is_ge
                )
                d12 = small.tile([P, 1], f32, tag="d12")
                nc.vector.tensor_sub(d12, m1, m2)
                d21 = small.tile([P, 1], f32, tag="d21")
                nc.vector.tensor_sub(d21, m2, m1)
                p1 = small.tile([P, 1], f32, tag="p1")
                nc.scalar.activation(p1, d12, AF.Sigmoid)
                p2 = small.tile([P, 1], f32, tag="p2")
                nc.scalar.activation(p2, d21, AF.Sigmoid)
                w1t = small.tile([P, E], f32, tag="w1t")
                nc.vector.tensor_scalar_mul(w1t, is1, p1)
                w2t = small.tile([P, E], f32, tag="w2t")
                nc.vector.tensor_scalar_mul(w2t, is2, p2)
                nc.vector.tensor_add(w_sb[:, tt, :], w1t, w2t)

            # ---- experts (dense, bf16) ----
            for e in range(E if PHASE >= 4 else 0):
                w1_sb = wpool.tile([P, CO, F], bf16, tag="w1")
                nc.sync.dma_start(
                    w1_sb, w1_d[e].rearrange("(co ci) f -> ci co f", ci=P)
                )
                w2_sb = wpool.tile([P, FO, C], bf16, tag="w2")
                nc.sync.dma_start(
                    w2_sb, w2_d[e].rearrange("(fo fi) c -> fi fo c", fi=P)
                )
                b1_sb = wpool.tile([P, FO], f32, tag="b1")
                nc.sync.dma_start(b1_sb, b1_d[e].rearrange("(o p) -> p o", p=P))
                b2_sb = wpool.tile([P, C], f32, tag="b2")
                nc.sync.dma_start(b2_sb, b2_d[e][None, :].to_broadcast((P, C)))
                for ch in range(tch_n):
                    hidT = hidp.tile([P, FO, 512], bf16, tag="hidT")
                    for ft in range(FO):
                        hps = ps_mm.tile([P, 512], f32, tag="mm512")
                        for co in range(CO):
                            nc.tensor.matmul(
                                hps,
                                w1_sb[:, co, ft * P : (ft + 1) * P],
                                h2T[:, co, ch * 512 : (ch + 1) * 512],
                                start=(co == 0),
                                stop=(co == CO - 1),
                            )
                        nc.scalar.activation(
                            hidT[:, ft, :], hps, AF.Relu,
                            bias=b1_sb[:, ft : ft + 1],
                        )
                    for ts in range(4):
                        tt = ch * 4 + ts
                        eo = ps_eo.tile([P, 512], f32, tag="eo")
                        for fo in range(FO):
                            nc.tensor.matmul(
                                eo,
                                hidT[:, fo, ts * P : (ts + 1) * P],
                                w2_sb[:, fo],
                                start=(fo == 0),
                                stop=(fo == FO - 1),
                            )
                        tmp = tmpp.tile([P, C], f32, tag="tmp")
                        nc.vector.tensor_add(tmp, eo, b2_sb)
                        tmp2 = tmpp.tile([P, C], f32, tag="tmp2")
                        nc.scalar.activation(
                            tmp2, tmp, AF.Copy, scale=w_sb[:, tt, e : e + 1]
                        )
                        if ACCUM:
                            nc.gpsimd.dma_start(
                                out_d[tt * P : (tt + 1) * P, :],
                                tmp2,
                                accum_op=OP.add,
                            )
                        else:
                            nc.sync.dma_start(
                                out_d[tt * P : (tt + 1) * P, :], tmp2
                            )

    nc.compile()
    return nc


def _get_nc(n_batches):
    if n_batches not in _CACHE:
        _CACHE[n_batches] = _build(n_batches)
    return _CACHE[n_batches]


def _make_mask():
    i = np.arange(P)[:, None]
    j = np.arange(2 * P)[None, :]
    return ((j > i) & (j <= i + WIN)).astype(np.float32)


def make_in_maps(x, weights, n_batches):
    import ml_dtypes

    mask = _make_mask()
    ntok = n_batches * T
    ncores = x.shape[0] // n_batches
    wmaps = {}
    for k, v in weights.items():
        v = np.asarray(v, np.float32)
        if k in ("W1", "W2"):
            v = v.astype(ml_dtypes.bfloat16)
        wmaps[k] = v
    wmaps["WgT"] = np.ascontiguousarray(wmaps.pop("Wg").T)
    in_maps = []
    for c in range(ncores):
        shard = np.ascontiguousarray(
            np.asarray(x[c * n_batches : (c + 1) * n_batches], np.float32).reshape(
                ntok, C
            )
        )
        m = {"x": shard, "mask": mask}
        m.update(wmaps)
        in_maps.append(m)
    return in_maps


def kernel(**inputs):
    from concourse import bass_utils

    nc = _get_nc(BPC)
    x = inputs["x"]
    weights = {k: v for k, v in inputs.items() if k != "x"}
    in_maps = make_in_maps(x, weights, BPC)
    res = bass_utils.run_bass_kernel_spmd(nc, in_maps, core_ids=list(range(NCORES)))
    out = np.concatenate(
        [res.results[c]["out"].reshape(BPC, T, C) for c in range(NCORES)], axis=0
    )
    return out
